# revision 14
# baseline (speedup 1.0000x reference)
"""DKT next-question BCE loss on 8 trn2 NeuronCores.

Data-parallel over students (32/core, 6368 valid rows + pad to 6400).
The loss touches ONE pred element per row (the one-hot row-dot), so the
HBM traffic floor is what decides performance. Batch ships bit-packed
(256B/row -> 1.6MB/core, a lossless re-encode of its exact 0.0/1.0
one-hot values) and pred ships as fp16 (13MB/core; the 2e-2 loss
tolerance leaves ~180x headroom for fp16 rounding, validated against
the f32 reference). All decoding happens on device, in 5 pipelined
1280-row windows:

1. XBAR transpose-load: packed batch rows [1280, 128 uint16] -> SBUF
   [128 words, 1280 rows]; word c of a row holds one-hot bits for
   elements j = 16c+t.
2. The idle tensor engine finds the one-hot position: words convert to
   fp16 (values 2^t exactly; DVE 4x tensor_copy) and two matmul columns
   [1, (c+1)/128] contract over the 128 word-partitions: F1 = 2^t,
   F2 = (c+1)/128 * 2^t, landing each row on its own PSUM partition.
3. f32 bit tricks decode (c, t) exactly on [128, 10] stats: t from F1's
   exponent field, 1/F1 = bitcast(0x7F000000 - bits(F1)),
   c = 128*F2*(1/F1) - 1, j = 16c+t, answer a = [j < 1024],
   qid = j mod 1024. All exact integer arithmetic in f32/int32.
   (The vector engine runs scalar_tensor_tensor at 1.33ns/elem with no
   fast mode, so wide per-row dots are avoided everywhere; only ops
   with 2x/4x DVE modes or the PE touch wide data.)
4. pred's window streams as fp16 [128, 10, 1024]; one 1024-wide
   iota-compare stt per column selects pred[r, qid] and accumulates
   into [128, 50] stats.
5. BCE tail once at the end: p clamped to [1e-6, 1-2^-11] (fp16 values
   near 1 round to exactly 1.0), log/log1p on the scalar engine,
   ll masked by [F1 > 0] so padded rows drop out. Host sums the 128x8
   partials (the all-reduce of the scalar loss) and negates.
"""

import sys

import numpy as np

sys.path.insert(0, "/opt/trn_rl_repo")

import concourse.bacc as bacc
import concourse.mybir as mybir
import concourse.tile as tile
from concourse.bass import IndirectOffsetOnAxis
from concourse.bass_utils import run_bass_kernel_spmd

B, T, Q = 256, 200, 1024
NCORES = 8
BS = B // NCORES              # students per core
ROWS = BS * (T - 1)           # 6368 valid rows per core
RPAD = 6400                   # padded rows
NW = 5                        # gather windows
WROWS = RPAD // NW            # 1280 rows per window
WCOLS = WROWS // 128          # 10 stat columns per window
NCOLS = NW * WCOLS            # 50

CLAMP_HI = 1.0 - 2.0 ** -11
CLAMP_LO = 1e-6

F32 = mybir.dt.float32
F16 = mybir.dt.float16
I32 = mybir.dt.int32
I16 = mybir.dt.int16
U16 = mybir.dt.uint16
_DEBUG_NO_GATHER = True
_cache: dict = {}


def _build():
    nc = bacc.Bacc("TRN2", target_bir_lowering=False, debug=False,
                   num_devices=NCORES)
    pred_h = nc.dram_tensor("pred", [RPAD, Q], F16, kind="ExternalInput")
    packed_h = nc.dram_tensor("packed", [RPAD, 128], U16, kind="ExternalInput")
    wmov_h = nc.dram_tensor("wmov", [128, 2], F16, kind="ExternalInput")
    rowb_h = nc.dram_tensor("rowbase", [128, NCOLS], I32, kind="ExternalInput")
    iota_h = nc.dram_tensor("iota64", [128, Q], F32, kind="ExternalInput")
    idn_h = nc.dram_tensor("idn", [128, 128], F16, kind="ExternalInput")
    out_h = nc.dram_tensor("out", [128, 1], F32, kind="ExternalOutput")

    mult = mybir.AluOpType.mult
    add = mybir.AluOpType.add
    Ln = mybir.ActivationFunctionType.Ln

    def ts(pool, in0, s1, op0, s2=None, op1=None, dtype=F32, tag="d"):
        o = pool.tile([128, WCOLS], dtype, tag=tag)
        kw = {"op1": op1} if op1 is not None else {}
        nc.vector.tensor_scalar(out=o[:], in0=in0, scalar1=s1, scalar2=s2,
                                op0=op0, **kw)
        return o

    with tile.TileContext(nc) as tc:
        with tc.tile_pool(name="const_p", bufs=1) as cp, \
             tc.tile_pool(name="x_p", bufs=3) as xp, \
             tc.tile_pool(name="xf_p", bufs=3) as fp, \
             tc.tile_pool(name="ps_p", bufs=4, space="PSUM") as psp, \
             tc.tile_pool(name="dec_p", bufs=3) as dp, \
             tc.tile_pool(name="idx_p", bufs=2) as ip, \
             tc.tile_pool(name="chunk_p", bufs=3) as ch, \
             tc.tile_pool(name="sel_p", bufs=2) as sp, \
             tc.tile_pool(name="acc_p", bufs=1) as ac:

            wmov = cp.tile([128, 2], F16)
            nc.sync.dma_start(out=wmov[:], in_=wmov_h[:, :])
            rowb = cp.tile([128, NCOLS], I32)
            nc.sync.dma_start(out=rowb[:], in_=rowb_h[:, :])
            iota = cp.tile([128, Q], F32)
            nc.sync.dma_start(out=iota[:], in_=iota_h[:, :])
            idn = cp.tile([128, 128], F16)
            nc.sync.dma_start(out=idn[:], in_=idn_h[:, :])

            psel = ac.tile([128, NCOLS], F32)
            aall = ac.tile([128, NCOLS], F32)
            mall = ac.tile([128, NCOLS], F32)
            qsall = ac.tile([128, NCOLS], F32)
            warm = ac.tile([128, 1], F32)
            nc.scalar.activation(warm[:], iota[:, 2:3], Ln)
            nc.scalar.activation(warm[:], iota[:, 0:1], Ln, bias=1.0,
                                 scale=-1.0)

            for w in range(NW):
                rows = slice(w * WROWS, (w + 1) * WROWS)
                # normal (fast) load: row r = 128c+p -> partition p
                xr = xp.tile([128, WCOLS, 128], U16, tag="xr")
                nc.sync.dma_start(
                    out=xr[:],
                    in_=packed_h[rows, :].rearrange("(c p) k -> p c k", p=128))
                xf = fp.tile([128, WCOLS, 128], F16, tag="xf")
                nc.vector.tensor_copy(xf[:], xr[:])

                # PE transpose per 128-row chunk (XBAR transpose-load is
                # ~23GB/s; PE does it nearly free), ACT copies PSUM->SBUF
                ps = psp.tile([128, WCOLS, 2], F32, tag="ps")
                for c in range(WCOLS):
                    tps = psp.tile([128, 128], F16, tag="tps")
                    nc.tensor.transpose(tps[:], xf[:, c, :], idn[:])
                    xs = fp.tile([128, 128], F16, tag="xs")
                    nc.scalar.copy(xs[:], tps[:])
                    nc.tensor.matmul(ps[:, c, :], xs[:],
                                     wmov[:], start=True, stop=True)
                F = dp.tile([128, WCOLS, 2], F32, tag="F")
                nc.vector.tensor_copy(F[:], ps[:])
                F1 = F[:, :, 0]
                F2 = F[:, :, 1]

                # mask: valid rows have a one-hot bit somewhere
                nc.vector.tensor_scalar(out=mall[:, w * WCOLS:(w + 1) * WCOLS],
                                        in0=F1, scalar1=0.0, scalar2=None,
                                        op0=mybir.AluOpType.is_gt)
                # exponent tricks: F1 = 2^t exactly
                e = F1.bitcast(I32)
                betab = ts(dp, e, 23, mybir.AluOpType.logical_shift_right,
                           dtype=I32, tag="betab")       # 127 + t
                invb = ts(dp, e, -1, mult, 0x7F000000, add, dtype=I32,
                          tag="invb")                     # bits of 1/F1
                betaf = dp.tile([128, WCOLS], F32, tag="betaf")
                nc.vector.tensor_copy(betaf[:], betab[:])  # 127+t as f32
                cc = dp.tile([128, WCOLS], F32, tag="cc")
                nc.vector.tensor_tensor(cc[:], F2, invb[:].bitcast(F32), mult)
                # j = 16*c + t = 2048*cc - 16 + (betaf - 127)
                j0 = dp.tile([128, WCOLS], F32, tag="j0")
                nc.vector.scalar_tensor_tensor(
                    out=j0[:], in0=cc[:], scalar=2048.0, in1=betaf[:],
                    op0=mult, op1=add)
                j = ts(dp, j0[:], -143.0, add, tag="j")
                ge = ts(dp, j[:], 1024.0, mybir.AluOpType.is_ge, tag="ge")
                # a = 1 - ge
                nc.vector.tensor_scalar(out=aall[:, w * WCOLS:(w + 1) * WCOLS],
                                        in0=ge[:], scalar1=-1.0, scalar2=1.0,
                                        op0=mult, op1=add)
                qid = dp.tile([128, WCOLS], F32, tag="qid")
                nc.vector.scalar_tensor_tensor(
                    out=qid[:], in0=ge[:], scalar=-1024.0, in1=j[:],
                    op0=mult, op1=add)
                nc.vector.tensor_scalar(
                    out=qsall[:, w * WCOLS:(w + 1) * WCOLS], in0=qid[:],
                    scalar1=1023.0, scalar2=0.0,
                    op0=mybir.AluOpType.min, op1=mybir.AluOpType.max)

            # Phase B: stream pred windows (fp16; these DMAs have no
            # upstream deps, so they prefetch from t=0 on the scalar
            # queue while phase A decodes) and select pred[r, qid] with
            # a 1024-wide iota-compare per column.
            for w in range(NW):
                rows = slice(w * WROWS, (w + 1) * WROWS)
                pw = ch.tile([128, WCOLS, Q], F16, tag="pw")
                nc.scalar.dma_start(
                    out=pw[:],
                    in_=pred_h[rows, :].rearrange("(c p) q -> p c q", p=128))
                for c in range(WCOLS):
                    col = w * WCOLS + c
                    junk = sp.tile([128, Q], F32, tag="junk")
                    nc.vector.scalar_tensor_tensor(
                        out=junk[:], in0=iota[:], scalar=qsall[:, col:col + 1],
                        in1=pw[:, c, :], op0=mybir.AluOpType.is_equal,
                        op1=mult,
                        accum_out=psel[:, col:col + 1])

            # BCE tail over all [128, NCOLS] stats.
            # fp16 pred values near 1 can round to exactly 1.0 (clamp HI
            # keeps log1p finite); padded rows have p=0 (clamp LO).
            spf = ac.tile([128, NCOLS], F32)
            nc.vector.tensor_scalar(out=spf[:], in0=psel[:],
                                    scalar1=CLAMP_HI, scalar2=CLAMP_LO,
                                    op0=mybir.AluOpType.min,
                                    op1=mybir.AluOpType.max)
            lp = ac.tile([128, NCOLS], F32)
            nc.scalar.activation(lp[:], spf[:], Ln)
            lq = ac.tile([128, NCOLS], F32)
            nc.scalar.activation(lq[:], spf[:], Ln, bias=1.0, scale=-1.0)
            d = ac.tile([128, NCOLS], F32)
            nc.vector.tensor_sub(d[:], lp[:], lq[:])
            ad = ac.tile([128, NCOLS], F32)
            nc.vector.tensor_mul(ad[:], aall[:], d[:])
            ll = ac.tile([128, NCOLS], F32)
            nc.vector.tensor_add(ll[:], lq[:], ad[:])
            llm = ac.tile([128, NCOLS], F32)
            nc.vector.tensor_mul(llm[:], ll[:], mall[:])
            part = ac.tile([128, 1], F32)
            nc.vector.tensor_reduce(out=part[:], in_=llm[:],
                                    axis=mybir.AxisListType.X,
                                    op=add)
            nc.sync.dma_start(out=out_h[:], in_=part[:])

    nc.compile()
    return nc


def _get_nc():
    if "nc" not in _cache:
        _cache["nc"] = _build()
    return _cache["nc"]


def _consts():
    c = np.arange(128, dtype=np.float32)
    wmov = np.stack([np.ones(128, np.float32), (c + 1.0) / 128.0],
                    axis=1).astype(np.float16)                     # [128, 2]
    p = np.arange(128, dtype=np.int32)[:, None]
    cidx = np.arange(NCOLS, dtype=np.int32)[None, :]
    rowbase = (16 * (128 * cidx + p)).astype(np.int32)             # [128, 50]
    iota64 = np.broadcast_to(np.arange(Q, dtype=np.float32),
                             (128, Q)).copy()                      # [128, 1024]
    idn = np.eye(128, dtype=np.float16)
    return wmov, rowbase, iota64, idn


def _in_maps(pred: np.ndarray, batch: np.ndarray) -> list[dict]:
    pred = np.asarray(pred)
    batch = np.asarray(batch)
    wmov, rowbase, iota64, idn = _consts()
    maps = []
    for cdev in range(NCORES):
        sl = slice(cdev * BS, (cdev + 1) * BS)
        pc = np.zeros((RPAD, Q), np.float16)
        pc[:ROWS] = pred[sl, :T - 1, :].reshape(ROWS, Q).astype(np.float16)
        bits = batch[sl, 1:, :].reshape(ROWS, 2 * Q) != 0.0
        pk = np.zeros((RPAD, 256), np.uint8)
        pk[:ROWS] = np.packbits(bits, axis=-1, bitorder="little")
        maps.append({"pred": pc, "packed": pk.view(np.uint16),
                     "wmov": wmov, "rowbase": rowbase, "iota64": iota64,
                     "idn": idn})
    return maps


def _axon_reset():
    """Best-effort device reset: clears wedged NRT state on the terminal
    left by previously crashed runs. No-op if the axon .so is absent."""
    try:
        import ctypes

        import jax
        jax.devices()
        lib = ctypes.CDLL("/opt/axon/libaxon_pjrt.so")
        lib.axon_reset.restype = ctypes.c_int64
        lib.axon_reset()
    except Exception:
        pass


def _run(pred: np.ndarray, batch: np.ndarray, trace: bool = False,
         all_cores: bool = False):
    nc = _get_nc()
    _axon_reset()
    kw = {"trace_cores": list(range(NCORES))} if all_cores else {}
    res = run_bass_kernel_spmd(nc, _in_maps(pred, batch),
                               list(range(NCORES)), trace=trace, **kw)
    total = np.sum([np.asarray(r["out"], np.float64).sum()
                    for r in res.results])
    loss = np.array([-total], dtype=np.float32)
    return loss, res


def kernel(pred: np.ndarray, batch: np.ndarray) -> np.ndarray:
    loss, _ = _run(pred, batch)
    return loss


# revision 15
# speedup vs baseline: 1.1340x; 1.1340x over previous
"""DKT next-question BCE loss on 8 trn2 NeuronCores.

Data-parallel over students (32/core, 6368 valid rows + pad to 6400).
The loss touches ONE pred element per row (the one-hot row-dot), so the
HBM traffic floor is what decides performance. Batch ships bit-packed
(256B/row -> 1.6MB/core, a lossless re-encode of its exact 0.0/1.0
one-hot values) and pred ships as fp16 (13MB/core; the 2e-2 loss
tolerance leaves ~180x headroom for fp16 rounding, validated against
the f32 reference). All decoding happens on device, in 5 pipelined
1280-row windows:

1. XBAR transpose-load: packed batch rows [1280, 128 uint16] -> SBUF
   [128 words, 1280 rows]; word c of a row holds one-hot bits for
   elements j = 16c+t.
2. The idle tensor engine finds the one-hot position: words convert to
   fp16 (values 2^t exactly; DVE 4x tensor_copy) and two matmul columns
   [1, (c+1)/128] contract over the 128 word-partitions: F1 = 2^t,
   F2 = (c+1)/128 * 2^t, landing each row on its own PSUM partition.
3. f32 bit tricks decode (c, t) exactly on [128, 10] stats: t from F1's
   exponent field, 1/F1 = bitcast(0x7F000000 - bits(F1)),
   c = 128*F2*(1/F1) - 1, j = 16c+t, answer a = [j < 1024],
   qid = j mod 1024. All exact integer arithmetic in f32/int32.
   (The vector engine runs scalar_tensor_tensor at 1.33ns/elem with no
   fast mode, so wide per-row dots are avoided everywhere; only ops
   with 2x/4x DVE modes or the PE touch wide data.)
4. pred's window streams as fp16 [128, 10, 1024]; one 1024-wide
   iota-compare stt per column selects pred[r, qid] and accumulates
   into [128, 50] stats.
5. BCE tail once at the end: p clamped to [1e-6, 1-2^-11] (fp16 values
   near 1 round to exactly 1.0), log/log1p on the scalar engine,
   ll masked by [F1 > 0] so padded rows drop out. Host sums the 128x8
   partials (the all-reduce of the scalar loss) and negates.
"""

import sys

import numpy as np

sys.path.insert(0, "/opt/trn_rl_repo")

import concourse.bacc as bacc
import concourse.mybir as mybir
import concourse.tile as tile
from concourse.bass import IndirectOffsetOnAxis
from concourse.bass_utils import run_bass_kernel_spmd

B, T, Q = 256, 200, 1024
NCORES = 8
BS = B // NCORES              # students per core
ROWS = BS * (T - 1)           # 6368 valid rows per core
RPAD = 6400                   # padded rows
NW = 5                        # gather windows
WROWS = RPAD // NW            # 1280 rows per window
WCOLS = WROWS // 128          # 10 stat columns per window
NCOLS = NW * WCOLS            # 50

CLAMP_HI = 1.0 - 2.0 ** -11
CLAMP_LO = 1e-6

F32 = mybir.dt.float32
F16 = mybir.dt.float16
I32 = mybir.dt.int32
I16 = mybir.dt.int16
U16 = mybir.dt.uint16
_DEBUG_NO_GATHER = True
_cache: dict = {}


def _build():
    nc = bacc.Bacc("TRN2", target_bir_lowering=False, debug=False,
                   num_devices=NCORES)
    pred_h = nc.dram_tensor("pred", [RPAD, Q], F16, kind="ExternalInput")
    packed_h = nc.dram_tensor("packed", [RPAD, 128], U16, kind="ExternalInput")
    wmov_h = nc.dram_tensor("wmov", [128, 2], F16, kind="ExternalInput")
    iota_h = nc.dram_tensor("iota64", [128, Q], F32, kind="ExternalInput")
    idn_h = nc.dram_tensor("idn", [128, 128], F16, kind="ExternalInput")
    out_h = nc.dram_tensor("out", [128, 1], F32, kind="ExternalOutput")

    mult = mybir.AluOpType.mult
    add = mybir.AluOpType.add
    Ln = mybir.ActivationFunctionType.Ln

    def ts(pool, in0, s1, op0, s2=None, op1=None, dtype=F32, tag="d"):
        o = pool.tile([128, WCOLS], dtype, tag=tag)
        kw = {"op1": op1} if op1 is not None else {}
        nc.vector.tensor_scalar(out=o[:], in0=in0, scalar1=s1, scalar2=s2,
                                op0=op0, **kw)
        return o

    with tile.TileContext(nc) as tc:
        with tc.tile_pool(name="const_p", bufs=1) as cp, \
             tc.tile_pool(name="x_p", bufs=3) as xp, \
             tc.tile_pool(name="xf_p", bufs=3) as fp, \
             tc.tile_pool(name="ps_p", bufs=4, space="PSUM") as psp, \
             tc.tile_pool(name="dec_p", bufs=3) as dp, \
             tc.tile_pool(name="idx_p", bufs=2) as ip, \
             tc.tile_pool(name="chunk_p", bufs=3) as ch, \
             tc.tile_pool(name="sel_p", bufs=2) as sp, \
             tc.tile_pool(name="acc_p", bufs=1) as ac:

            wmov = cp.tile([128, 2], F16)
            nc.sync.dma_start(out=wmov[:], in_=wmov_h[:, :])
            iota = cp.tile([128, Q], F32)
            nc.sync.dma_start(out=iota[:], in_=iota_h[:, :])
            idn = cp.tile([128, 128], F16)
            nc.sync.dma_start(out=idn[:], in_=idn_h[:, :])

            psel = ac.tile([128, NCOLS], F32)
            aall = ac.tile([128, NCOLS], F32)
            mall = ac.tile([128, NCOLS], F32)
            qsall = ac.tile([128, NCOLS], F32)
            warm = ac.tile([128, 1], F32)
            nc.scalar.activation(warm[:], iota[:, 2:3], Ln)
            nc.scalar.activation(warm[:], iota[:, 0:1], Ln, bias=1.0,
                                 scale=-1.0)

            for w in range(NW):
                rows = slice(w * WROWS, (w + 1) * WROWS)
                # normal (fast) load: row r = 128c+p -> partition p
                xr = xp.tile([128, WCOLS, 128], U16, tag="xr")
                nc.sync.dma_start(
                    out=xr[:],
                    in_=packed_h[rows, :].rearrange("(p c) k -> p c k", p=128))
                xf = fp.tile([128, WCOLS, 128], F16, tag="xf")
                nc.vector.tensor_copy(xf[:], xr[:])

                # PE transpose per 128-row chunk (XBAR transpose-load is
                # ~23GB/s; PE does it nearly free), ACT copies PSUM->SBUF
                ps = psp.tile([128, WCOLS, 2], F32, tag="ps")
                for c in range(WCOLS):
                    tps = psp.tile([128, 128], F16, tag="tps")
                    nc.tensor.transpose(tps[:], xf[:, c, :], idn[:])
                    xs = fp.tile([128, 128], F16, tag="xs")
                    nc.scalar.copy(xs[:], tps[:])
                    nc.tensor.matmul(ps[:, c, :], xs[:],
                                     wmov[:], start=True, stop=True)
                F = dp.tile([128, WCOLS, 2], F32, tag="F")
                nc.vector.tensor_copy(F[:], ps[:])
                F1 = F[:, :, 0]
                F2 = F[:, :, 1]

                # mask: valid rows have a one-hot bit somewhere
                nc.vector.tensor_scalar(out=mall[:, w * WCOLS:(w + 1) * WCOLS],
                                        in0=F1, scalar1=0.0, scalar2=None,
                                        op0=mybir.AluOpType.is_gt)
                # exponent tricks: F1 = 2^t exactly
                e = F1.bitcast(I32)
                betab = ts(dp, e, 23, mybir.AluOpType.logical_shift_right,
                           dtype=I32, tag="betab")       # 127 + t
                invb = ts(dp, e, -1, mult, 0x7F000000, add, dtype=I32,
                          tag="invb")                     # bits of 1/F1
                betaf = dp.tile([128, WCOLS], F32, tag="betaf")
                nc.vector.tensor_copy(betaf[:], betab[:])  # 127+t as f32
                cc = dp.tile([128, WCOLS], F32, tag="cc")
                nc.vector.tensor_tensor(cc[:], F2, invb[:].bitcast(F32), mult)
                # j = 16*c + t = 2048*cc - 16 + (betaf - 127)
                j0 = dp.tile([128, WCOLS], F32, tag="j0")
                nc.vector.scalar_tensor_tensor(
                    out=j0[:], in0=cc[:], scalar=2048.0, in1=betaf[:],
                    op0=mult, op1=add)
                j = ts(dp, j0[:], -143.0, add, tag="j")
                ge = ts(dp, j[:], 1024.0, mybir.AluOpType.is_ge, tag="ge")
                # a = 1 - ge
                nc.vector.tensor_scalar(out=aall[:, w * WCOLS:(w + 1) * WCOLS],
                                        in0=ge[:], scalar1=-1.0, scalar2=1.0,
                                        op0=mult, op1=add)
                qid = dp.tile([128, WCOLS], F32, tag="qid")
                nc.vector.scalar_tensor_tensor(
                    out=qid[:], in0=ge[:], scalar=-1024.0, in1=j[:],
                    op0=mult, op1=add)
                nc.vector.tensor_scalar(
                    out=qsall[:, w * WCOLS:(w + 1) * WCOLS], in0=qid[:],
                    scalar1=1023.0, scalar2=0.0,
                    op0=mybir.AluOpType.min, op1=mybir.AluOpType.max)

            # Phase B: stream pred windows (fp16; these DMAs have no
            # upstream deps, so they prefetch from t=0 on the scalar
            # queue while phase A decodes) and select pred[r, qid] with
            # a 1024-wide iota-compare per column.
            for w in range(NW):
                rows = slice(w * WROWS, (w + 1) * WROWS)
                pw = ch.tile([128, WCOLS, Q], F16, tag="pw")
                pv = pred_h[rows, :].rearrange("(p c) q -> p c q", p=128)
                h = WCOLS // 2
                nc.sync.dma_start(out=pw[:, :h, :], in_=pv[:, :h, :])
                nc.sync.dma_start(out=pw[:, h:, :], in_=pv[:, h:, :])
                for c in range(WCOLS):
                    col = w * WCOLS + c
                    junk = sp.tile([128, Q], F32, tag="junk")
                    nc.vector.scalar_tensor_tensor(
                        out=junk[:], in0=iota[:], scalar=qsall[:, col:col + 1],
                        in1=pw[:, c, :], op0=mybir.AluOpType.is_equal,
                        op1=mult,
                        accum_out=psel[:, col:col + 1])

            # BCE tail over all [128, NCOLS] stats.
            # fp16 pred values near 1 can round to exactly 1.0 (clamp HI
            # keeps log1p finite); padded rows have p=0 (clamp LO).
            spf = ac.tile([128, NCOLS], F32)
            nc.vector.tensor_scalar(out=spf[:], in0=psel[:],
                                    scalar1=CLAMP_HI, scalar2=CLAMP_LO,
                                    op0=mybir.AluOpType.min,
                                    op1=mybir.AluOpType.max)
            lp = ac.tile([128, NCOLS], F32)
            nc.scalar.activation(lp[:], spf[:], Ln)
            lq = ac.tile([128, NCOLS], F32)
            nc.scalar.activation(lq[:], spf[:], Ln, bias=1.0, scale=-1.0)
            d = ac.tile([128, NCOLS], F32)
            nc.vector.tensor_sub(d[:], lp[:], lq[:])
            ad = ac.tile([128, NCOLS], F32)
            nc.vector.tensor_mul(ad[:], aall[:], d[:])
            ll = ac.tile([128, NCOLS], F32)
            nc.vector.tensor_add(ll[:], lq[:], ad[:])
            llm = ac.tile([128, NCOLS], F32)
            nc.vector.tensor_mul(llm[:], ll[:], mall[:])
            part = ac.tile([128, 1], F32)
            nc.vector.tensor_reduce(out=part[:], in_=llm[:],
                                    axis=mybir.AxisListType.X,
                                    op=add)
            nc.sync.dma_start(out=out_h[:], in_=part[:])

    nc.compile()
    return nc


def _get_nc():
    if "nc" not in _cache:
        _cache["nc"] = _build()
    return _cache["nc"]


def _consts():
    c = np.arange(128, dtype=np.float32)
    wmov = np.stack([np.ones(128, np.float32), (c + 1.0) / 128.0],
                    axis=1).astype(np.float16)                     # [128, 2]
    iota64 = np.broadcast_to(np.arange(Q, dtype=np.float32),
                             (128, Q)).copy()                      # [128, 1024]
    idn = np.eye(128, dtype=np.float16)
    return wmov, iota64, idn


def _in_maps(pred: np.ndarray, batch: np.ndarray) -> list[dict]:
    pred = np.asarray(pred)
    batch = np.asarray(batch)
    wmov, iota64, idn = _consts()
    maps = []
    for cdev in range(NCORES):
        sl = slice(cdev * BS, (cdev + 1) * BS)
        pc = np.zeros((RPAD, Q), np.float16)
        pc[:ROWS] = pred[sl, :T - 1, :].reshape(ROWS, Q).astype(np.float16)
        bits = batch[sl, 1:, :].reshape(ROWS, 2 * Q) != 0.0
        pk = np.zeros((RPAD, 256), np.uint8)
        pk[:ROWS] = np.packbits(bits, axis=-1, bitorder="little")
        maps.append({"pred": pc, "packed": pk.view(np.uint16),
                     "wmov": wmov, "iota64": iota64, "idn": idn})
    return maps


def _axon_reset():
    """Best-effort device reset: clears wedged NRT state on the terminal
    left by previously crashed runs. No-op if the axon .so is absent."""
    try:
        import ctypes

        import jax
        jax.devices()
        lib = ctypes.CDLL("/opt/axon/libaxon_pjrt.so")
        lib.axon_reset.restype = ctypes.c_int64
        lib.axon_reset()
    except Exception:
        pass


def _run(pred: np.ndarray, batch: np.ndarray, trace: bool = False,
         all_cores: bool = False):
    nc = _get_nc()
    _axon_reset()
    kw = {"trace_cores": list(range(NCORES))} if all_cores else {}
    res = run_bass_kernel_spmd(nc, _in_maps(pred, batch),
                               list(range(NCORES)), trace=trace, **kw)
    total = np.sum([np.asarray(r["out"], np.float64).sum()
                    for r in res.results])
    loss = np.array([-total], dtype=np.float32)
    return loss, res


def kernel(pred: np.ndarray, batch: np.ndarray) -> np.ndarray:
    loss, _ = _run(pred, batch)
    return loss
